# revision 28
# baseline (speedup 1.0000x reference)
"""MetaLSTMCell Trainium2 kernel: 8 cores on a (batch x 2, hidden x 4) grid.

Core i handles batch rows bi*1024:(bi+1)*1024 (bi = i//4) and hidden columns
hi*256:(hi+1)*256 (hi = i%4) for all 4 gates.

v2 design (after trace analysis of the v1 baseline):
- ALL heavy preprocessing on host: gate permute to [i,f,o,g], hypernetwork
  fold M_* = d*_w @ z*_w, bias folds, per-core slicing/transposes, and bf16
  casts.  Device receives DMA-ready bf16 tiles (halves DMA bytes, removes
  every on-device cast that was stalling the PE).
- Inputs fully resident in SBUF (one big DMA each) so the MM stream has no
  per-tile DMA dependencies -> PE stays dense/warm (v1 ran at 1.2 GHz cold
  nearly all kernel: 241us throttled).
- Fused DVE ops: tensor_tensor_reduce produces y and the per-gate sums (and
  y^2 sums) in single passes; affine_mul_reduce applies (y-mu)*rs*ln_w in one
  op per gate.
- rsqrt(var+eps) via int bit-trick + 2 Newton steps on VectorE: the v1
  scalar-engine Sqrt forced ~2 activation-table reloads (~2.7us each) per
  batch tile (no table set holds both Sqrt and Sigmoid/Tanh).
- 4 grouped AllReduces (tiles 0-2 / 3-4 / 5-6 / 7, issued at bt 2/4/6/7)
  instead of 16 tiny ones: v1's collectives serialized at 6-36us each and
  gated a 107us tail; the final AR covers a single tile so it gates minimal
  tail work. (Measured: 2 bigger ARs regress — phase_b piles into the tail.)
- bf16 outputs (host upcasts); rel-err budget is 2e-2, measured 3.8e-3.

Measured on HW: 195us vs 289us baseline (1.48x), rel err 3.79e-3.
"""

import sys

sys.path.insert(0, "/opt/trn_rl_repo")

import numpy as np
import ml_dtypes
import concourse.bass as bass
import concourse.mybir as mybir
import concourse.tile as tile
from concourse.bass_utils import run_bass_kernel_spmd

B, IN, H, Z, G = 2048, 1024, 1024, 256, 4
NCORES = 8
BI_W, HI_W = 2, 4          # core grid: batch ways x hidden ways
BSH = B // BI_W            # 1024 batch rows per core
HSH = H // HI_W            # 256 hidden cols per core
HS = 128                   # h-subtile width
NHU = HSH // HS            # 2 h-subtiles per core
N = G * HS                 # 512: unit column width (4 gates x 128)
BT = 128                   # batch tile
NBT = BSH // BT            # 8 batch tiles per core
KC = IN // 128             # 8 K-chunks for the main GEMMs
ZC = Z // 128              # 2 K-chunks for the D GEMMs
PERM = (0, 1, 3, 2)        # gate order [i, f, o, g]
GRPS = ((0, 3), (3, 5), (5, 7), (7, 8))   # AllReduce tile groups
MAGIC = 0x5F375A86         # rsqrt seed constant
BF = ml_dtypes.bfloat16

dt = mybir.dt
AF = mybir.ActivationFunctionType
ALU = mybir.AluOpType
F32, BF16, I32 = dt.float32, dt.bfloat16, dt.int32


def fixup_multi_waits(nc):
    """This toolchain's walrus accepts at most ONE sync wait per instruction;
    Tile emits several. Hoist extras onto same-engine NOPs placed before."""
    for f in nc.m.functions:
        for blk in f.blocks:
            out = []
            changed = False
            for inst in blk.instructions:
                si = getattr(inst, "sync_info", None)
                waits = list(si.on_wait) if si is not None and si.on_wait else []
                if len(waits) > 1:
                    changed = True
                    for k, w in enumerate(waits[:-1]):
                        nop = mybir.InstNoOp(
                            name=f"{inst.name}-waitsplit{k}", ins=[], outs=[]
                        )
                        nop.engine = inst.engine
                        nop.sync_info = mybir.SyncInfo(on_wait=[w], on_update=[])
                        out.append(nop)
                    si.on_wait = [waits[-1]]
                out.append(inst)
            if changed:
                blk.instructions = out


def build(apply_fixup=True):
    nc = bass.Bass(trn_type="TRN2", num_devices=NCORES)
    P = 128

    def din(name, shape, dtype=BF16):
        return nc.dram_tensor(name, shape, dtype, kind="ExternalInput")

    xtt = din("xtt", [P, KC, NBT, BT])
    htt = din("htt", [P, KC, NBT, BT])
    mtt = din("mtt", [P, ZC, NBT, BT])
    ctt = din("ctt", [P, NBT, HSH])
    whbD = din("whbD", [P, NHU, KC, N])
    wxbD = din("wxbD", [P, NHU, KC, N])
    MhD = din("MhD", [P, NHU, ZC, N])
    MxD = din("MxD", [P, NHU, ZC, N])
    MbD = din("MbD", [P, NHU, ZC, N])
    rowsD = din("rowsD", [1, 3, NHU, N])
    lnwD = din("lnwD", [P, NHU, N])
    lnbD = din("lnbD", [P, NHU, N])
    hn = nc.dram_tensor("hn", [BSH, HSH], BF16, kind="ExternalOutput")
    cn = nc.dram_tensor("cn", [BSH, HSH], BF16, kind="ExternalOutput")

    quad_groups = [[0, 1, 2, 3], [4, 5, 6, 7]]

    with tile.TileContext(nc) as tc:
        with tc.tile_pool(name="wres", bufs=1) as wres, \
             tc.tile_pool(name="dram", bufs=1, space="DRAM") as dram, \
             tc.tile_pool(name="ev", bufs=3) as ev, \
             tc.tile_pool(name="mp", bufs=3) as mp, \
             tc.tile_pool(name="yp", bufs=12) as yp, \
             tc.tile_pool(name="gp", bufs=1) as gp, \
             tc.tile_pool(name="pb", bufs=3) as pb, \
             tc.tile_pool(name="ot", bufs=6) as ot, \
             tc.tile_pool(name="psd", bufs=4, space="PSUM") as psd, \
             tc.tile_pool(name="psh", bufs=2, space="PSUM") as psh, \
             tc.tile_pool(name="psx", bufs=2, space="PSUM") as psx:

            mom_in = dram.tile([BSH, 8], F32)
            mom_out = dram.tile([BSH, 8], F32)
            warm_in = dram.tile([1, 8], F32)
            warm_out = dram.tile([1, 8], F32)

            # warm-up collective: absorbs the CC entry barrier while the
            # weight DMAs stream in
            wz = wres.tile([1, 8], F32)
            nc.vector.memset(wz[:], 0.0)
            nc.sync.dma_start(warm_in[:], wz[:])
            nc.gpsimd.collective_compute(
                "AllReduce", ALU.add, replica_groups=quad_groups,
                ins=[warm_in[:]], outs=[warm_out[:]])

            # ---- persistent tiles / preamble DMAs (ordered so the D-GEMM
            # and WH-GEMM operands land first)
            Mh = wres.tile([P, NHU, ZC, N], BF16)
            Mx = wres.tile([P, NHU, ZC, N], BF16)
            Mb = wres.tile([P, NHU, ZC, N], BF16)
            mball = wres.tile([P, ZC, NBT, BT], BF16)
            b3 = wres.tile([P, 3, NHU, N], BF16)
            nc.vector.memset(b3[:], 0.0)
            e0 = wres.tile([P, P], BF16)
            nc.vector.memset(e0[:], 0.0)
            nc.vector.memset(e0[:1, :], 1.0)
            magic_t = wres.tile([P, 4, G], I32)
            nc.vector.memset(magic_t[:], MAGIC)
            one_t = wres.tile([P, 4, G], I32)
            nc.vector.memset(one_t[:], 1)

            nc.sync.dma_start(mball[:], mtt.ap())
            nc.sync.dma_start(Mh[:], MhD.ap())
            nc.sync.dma_start(Mx[:], MxD.ap())
            nc.sync.dma_start(Mb[:], MbD.ap())
            nc.sync.dma_start(b3[0:1], rowsD.ap())

            hball = wres.tile([P, KC, NBT, BT], BF16)
            nc.sync.dma_start(hball[:], htt.ap())
            whb = wres.tile([P, NHU, KC, N], BF16)
            for hu in range(NHU):
                nc.sync.dma_start(whb[:, hu], whbD.ap()[:, hu])
            xball = wres.tile([P, KC, NBT, BT], BF16)
            nc.sync.dma_start(xball[:], xtt.ap())
            wxb = wres.tile([P, NHU, KC, N], BF16)
            for hu in range(NHU):
                nc.sync.dma_start(wxb[:, hu], wxbD.ap()[:, hu])
            call = wres.tile([P, NBT, HSH], BF16)
            nc.sync.dma_start(call[:], ctt.ap())
            lnw_r = wres.tile([P, NHU, N], BF16)
            nc.sync.dma_start(lnw_r[:], lnwD.ap())
            lnb_r = wres.tile([P, NHU, N], BF16)
            nc.sync.dma_start(lnb_r[:], lnbD.ap())

            ytiles = {}
            obuf = {}
            rsg = {}
            nmg = {}

            def phase_a(bt):
                bs = slice(bt * BT, (bt + 1) * BT)
                mom = mp.tile([P, 8], F32, tag="mom")
                pm = mp.tile([P, 8], F32, tag="pm")
                obuf[bt] = (ot.tile([P, HSH], BF16, tag="cnb", name="cnb"),
                            ot.tile([P, HSH], BF16, tag="hnb", name="hnb"))
                for hu in range(NHU):
                    DH = psd.tile([P, N], F32, tag="psd")
                    DX = psd.tile([P, N], F32, tag="psd")
                    DB = psd.tile([P, N], F32, tag="psd")
                    for (D, MT, j) in ((DH, Mh, 0), (DX, Mx, 1), (DB, Mb, 2)):
                        for kc in range(ZC):
                            nc.tensor.matmul(D[:], mball[:, kc, bt],
                                             MT[:, hu, kc],
                                             start=(kc == 0), stop=False)
                        nc.tensor.matmul(D[:], e0[:], b3[:, j, hu],
                                         start=False, stop=True)
                    WH = psh.tile([P, N], F32, tag="psh")
                    for kc in range(KC):
                        nc.tensor.matmul(WH[:], hball[:, kc, bt],
                                         whb[:, hu, kc], start=(kc == 0),
                                         stop=(kc == KC - 1))
                    WX = psx.tile([P, N], F32, tag="psx")
                    for kc in range(KC):
                        nc.tensor.matmul(WX[:], xball[:, kc, bt],
                                         wxb[:, hu, kc], start=(kc == 0),
                                         stop=(kc == KC - 1))

                    dh_s = ev.tile([P, N], BF16, tag="dh_s")
                    nc.scalar.copy(dh_s[:], DH[:])
                    dx_s = ev.tile([P, N], BF16, tag="dx_s")
                    nc.scalar.copy(dx_s[:], DX[:])
                    db_s = ev.tile([P, N], BF16, tag="db_s")
                    nc.scalar.copy(db_s[:], DB[:])
                    wh_s = ev.tile([P, N], BF16, tag="wh_s")
                    nc.scalar.copy(wh_s[:], WH[:])
                    wx_s = ev.tile([P, N], BF16, tag="wx_s")
                    nc.scalar.copy(wx_s[:], WX[:])

                    y1 = ev.tile([P, N], BF16, tag="y1")
                    nc.gpsimd.tensor_mul(y1[:], wh_s[:], dh_s[:])
                    y2 = ev.tile([P, N], BF16, tag="y2")
                    nc.gpsimd.tensor_mul(y2[:], wx_s[:], dx_s[:])
                    y12 = ev.tile([P, N], BF16, tag="y12")
                    nc.vector.tensor_add(y12[:], y1[:], y2[:])

                    y = yp.tile([P, N], BF16, tag="y")
                    ysq = ev.tile([P, N], BF16, tag="ysq")
                    acc = pm if hu == 0 else mom
                    for g in range(G):
                        gs = slice(g * HS, (g + 1) * HS)
                        nc.vector.scalar_tensor_tensor(
                            y[:, gs], y12[:, gs], 1.0, db_s[:, gs],
                            ALU.mult, ALU.add, accum_out=acc[:, g:g + 1])
                    for g in range(G):
                        gs = slice(g * HS, (g + 1) * HS)
                        nc.vector.scalar_tensor_tensor(
                            ysq[:, gs], y[:, gs], 1.0, y[:, gs],
                            ALU.mult, ALU.mult, accum_out=acc[:, 4 + g:5 + g])
                    ytiles[(bt, hu)] = y
                nc.vector.tensor_add(mom[:], mom[:], pm[:])
                nc.sync.dma_start(mom_in[bs, :], mom[:])

            def ar(grp):
                t0, t1 = GRPS[grp]
                rows = slice(t0 * BT, t1 * BT)
                nc.gpsimd.collective_compute(
                    "AllReduce", ALU.add, replica_groups=quad_groups,
                    ins=[mom_in[rows, :]], outs=[mom_out[rows, :]])

            def prep(grp):
                t0, t1 = GRPS[grp]
                nt = t1 - t0
                gm = gp.tile([P, nt, 8], F32, tag=f"gm{grp}")
                nc.sync.dma_start(
                    gm[:], mom_out[t0 * BT:t1 * BT, :]
                    .rearrange("(t p) c -> p t c", p=P))
                scl = gp.tile([P, nt, 8], F32, tag=f"scl{grp}")
                nc.vector.tensor_scalar_mul(scl[:], gm[:], 1.0 / H)
                mu = scl[:, :, 0:4]
                msq = scl[:, :, 4:8]
                musq = gp.tile([P, nt, G], F32, tag=f"musq{grp}")
                nc.vector.tensor_mul(musq[:], mu, mu)
                veps = gp.tile([P, nt, G], F32, tag=f"veps{grp}")
                nc.vector.scalar_tensor_tensor(
                    veps[:], musq[:], -1.0, msq, ALU.mult, ALU.add)
                nc.vector.tensor_scalar_add(veps[:], veps[:], 1e-5)
                sh = gp.tile([P, nt, G], I32, tag=f"sh{grp}")
                nc.vector.tensor_tensor(sh[:], veps[:].bitcast(I32),
                                        one_t[:, :nt], ALU.logical_shift_right)
                x = gp.tile([P, nt, G], F32, tag=f"x{grp}")
                nc.vector.tensor_tensor(x[:].bitcast(I32), magic_t[:, :nt],
                                        sh[:], ALU.subtract)
                a = gp.tile([P, nt, G], F32, tag=f"a{grp}")
                b2 = gp.tile([P, nt, G], F32, tag=f"b2{grp}")
                for _ in range(2):
                    nc.vector.tensor_mul(a[:], x[:], x[:])
                    nc.vector.tensor_mul(b2[:], a[:], veps[:])
                    nc.vector.tensor_scalar(b2[:], b2[:], -0.5, 1.5,
                                            op0=ALU.mult, op1=ALU.add)
                    nc.vector.tensor_mul(x[:], x[:], b2[:])
                nmt = gp.tile([P, nt, G], F32, tag=f"nmt{grp}")
                nc.vector.scalar_tensor_tensor(
                    nmt[:], mu, -1.0, x[:], ALU.mult, ALU.mult)
                rsg[grp] = x
                nmg[grp] = nmt

            def phase_b(bt):
                grp = next(g for g, (t0, t1) in enumerate(GRPS)
                           if t0 <= bt < t1)
                gi = bt - GRPS[grp][0]
                rs = rsg[grp]
                nm = nmg[grp]
                cn_bt, hn_bt = obuf.pop(bt)
                bs = slice(bt * BT, (bt + 1) * BT)
                for hu in range(NHU):
                    y = ytiles.pop((bt, hu))
                    u = pb.tile([P, N], BF16, tag="u")
                    for g in range(G):
                        gs = slice(g * HS, (g + 1) * HS)
                        nc.vector.tensor_scalar(
                            u[:, gs], y[:, gs], rs[:, gi, g:g + 1],
                            nm[:, gi, g:g + 1], op0=ALU.mult, op1=ALU.add)
                    vv = pb.tile([P, N], BF16, tag="vv")
                    nc.vector.tensor_mul(vv[:], u[:], lnw_r[:, hu])
                    vv2 = pb.tile([P, N], BF16, tag="vv2")
                    nc.vector.tensor_add(vv2[:], vv[:], lnb_r[:, hu])
                    gt = pb.tile([P, N], BF16, tag="gt")
                    nc.scalar.activation(gt[:, 0:3 * HS], vv2[:, 0:3 * HS],
                                         AF.Sigmoid)
                    nc.scalar.activation(gt[:, 3 * HS:N], vv2[:, 3 * HS:N],
                                         AF.Tanh)
                    hs_cols = slice(hu * HS, (hu + 1) * HS)
                    cs = call[:, bt, hs_cols]
                    sfc = pb.tile([P, HS], BF16, tag="sfc")
                    nc.gpsimd.tensor_mul(sfc[:], gt[:, HS:2 * HS], cs)
                    sit = pb.tile([P, HS], BF16, tag="sit")
                    nc.gpsimd.tensor_mul(sit[:], gt[:, 0:HS], gt[:, 3 * HS:N])
                    nc.gpsimd.tensor_add(cn_bt[:, hs_cols], sfc[:], sit[:])
                    tc_t = pb.tile([P, HS], BF16, tag="tc")
                    nc.scalar.activation(tc_t[:], cn_bt[:, hs_cols], AF.Tanh)
                    nc.gpsimd.tensor_mul(hn_bt[:, hs_cols],
                                         gt[:, 2 * HS:3 * HS], tc_t[:])
                nc.sync.dma_start(cn[bs, :], cn_bt[:])
                nc.sync.dma_start(hn[bs, :], hn_bt[:])

            # ---- main schedule: ARs at bt 2/4/6/7; last group is a single
            # tile so the final collective gates minimal tail work
            for bt in range(NBT):
                phase_a(bt)
                if bt == 2:
                    ar(0)
                if bt == 4:
                    ar(1)
                if bt == 6:
                    ar(2)
                if bt == 7:
                    ar(3)
                if bt == 4:
                    prep(0)
                if bt == 6:
                    prep(1)
                if bt >= 4 and bt <= 6:
                    phase_b(bt - 4)
            phase_b(3)
            phase_b(4)
            prep(2)
            phase_b(5)
            phase_b(6)
            prep(3)
            phase_b(7)

    if apply_fixup:
        fixup_multi_waits(nc)
    return nc


_nc = None


def _get_nc():
    global _nc
    if _nc is None:
        _nc = build()
    return _nc


def make_in_maps(src_x, h, c, src_meta, zh_w, zh_b, zx_w, zx_b, zb_w,
                 dh_w, dx_w, db_w, db_b, w_h, w_x, ln_w, ln_b):
    f32 = np.float32
    asc = np.ascontiguousarray
    perm = list(PERM)
    w_h = w_h[perm]
    w_x = w_x[perm]
    dh_w = dh_w[perm]
    dx_w = dx_w[perm]
    db_w = db_w[perm]
    db_b = db_b[perm]
    ln_w = ln_w[perm]
    ln_b = ln_b[perm]
    zh_w3 = zh_w.reshape(G, Z, Z)[perm]
    zx_w3 = zx_w.reshape(G, Z, Z)[perm]
    zb_w3 = zb_w.reshape(G, Z, Z)[perm]
    zh_b2 = zh_b.reshape(G, Z)[perm]
    zx_b2 = zx_b.reshape(G, Z)[perm]

    # hypernetwork fold: M*[g,h,z'] = sum_z d*_w[g,h,z] * z*_w[g,z,z']
    Mh_full = np.matmul(dh_w, zh_w3)          # [G, H, Z]
    Mx_full = np.matmul(dx_w, zx_w3)
    Mb_full = np.matmul(db_w, zb_w3)

    xT = asc(src_x.T.astype(f32, copy=False))  # [IN, B]
    hT = asc(h.T.astype(f32, copy=False))
    mT = asc(src_meta.T.astype(f32, copy=False))

    def act_tiles(aT, brows, kchunks):
        # [K, BSH] -> [128p, kc, bt, bb] bf16
        sl = aT[:, brows]
        return asc(sl.reshape(kchunks, 128, NBT, BT)
                   .transpose(1, 0, 2, 3).astype(BF))

    def per_hu_w(w):
        # [G, HSH, IN] slice -> [128p, NHU, KC, N] bf16
        out = np.empty((NHU, IN, N), f32)
        for hu in range(NHU):
            blk = w[:, hu * HS:(hu + 1) * HS, :]   # [G, HS, IN]
            out[hu] = blk.transpose(2, 0, 1).reshape(IN, N)
        return asc(out.reshape(NHU, KC, 128, N)
                   .transpose(2, 0, 1, 3).astype(BF))

    def per_hu_m(Mfull_sl):
        # [G, HSH, Z] slice -> [128p, NHU, ZC, N] bf16
        out = np.empty((NHU, Z, N), f32)
        for hu in range(NHU):
            blk = Mfull_sl[:, hu * HS:(hu + 1) * HS, :]  # [G, HS, Z]
            out[hu] = blk.transpose(2, 0, 1).reshape(Z, N)
        return asc(out.reshape(NHU, ZC, 128, N)
                   .transpose(2, 0, 1, 3).astype(BF))

    def per_hu_row(v):
        # v: [G, HSH] -> [NHU, N] with [hu][g*HS+hh]
        return (v.reshape(G, NHU, HS).transpose(1, 0, 2)
                .reshape(NHU, N).astype(f32))

    in_maps = []
    for ci in range(NCORES):
        bi, hi = ci // HI_W, ci % HI_W
        brows = slice(bi * BSH, (bi + 1) * BSH)
        hcols = slice(hi * HSH, (hi + 1) * HSH)

        bdh_c = np.einsum("gz,ghz->gh", zh_b2, dh_w[:, hcols, :])
        bdx_c = np.einsum("gz,ghz->gh", zx_b2, dx_w[:, hcols, :])
        rows3 = np.stack([per_hu_row(bdh_c), per_hu_row(bdx_c),
                          per_hu_row(db_b[:, hcols])])  # [3, NHU, N]

        cb = c[brows, hcols].reshape(NBT, 128, HSH).transpose(1, 0, 2)

        in_maps.append({
            "xtt": act_tiles(xT, brows, KC),
            "htt": act_tiles(hT, brows, KC),
            "mtt": act_tiles(mT, brows, ZC),
            "ctt": asc(cb.astype(BF)),
            "whbD": per_hu_w(w_h[:, hcols, :]),
            "wxbD": per_hu_w(w_x[:, hcols, :]),
            "MhD": per_hu_m(Mh_full[:, hcols, :]),
            "MxD": per_hu_m(Mx_full[:, hcols, :]),
            "MbD": per_hu_m(Mb_full[:, hcols, :]),
            "rowsD": asc(rows3[None].astype(BF)),
            "lnwD": asc(np.broadcast_to(per_hu_row(ln_w[:, hcols])[None],
                                        (128, NHU, N)).astype(BF)),
            "lnbD": asc(np.broadcast_to(per_hu_row(ln_b[:, hcols])[None],
                                        (128, NHU, N)).astype(BF)),
        })
    return in_maps


def run(inputs, trace=False):
    nc = _get_nc()
    in_maps = make_in_maps(**inputs)
    res = run_bass_kernel_spmd(nc, in_maps, core_ids=list(range(NCORES)),
                               trace=trace)
    h_next = np.empty((B, H), np.float32)
    c_next = np.empty((B, H), np.float32)
    for ci in range(NCORES):
        bi, hi = ci // HI_W, ci % HI_W
        brows = slice(bi * BSH, (bi + 1) * BSH)
        hcols = slice(hi * HSH, (hi + 1) * HSH)
        h_next[brows, hcols] = np.asarray(res.results[ci]["hn"]).astype(
            np.float32)
        c_next[brows, hcols] = np.asarray(res.results[ci]["cn"]).astype(
            np.float32)
    return (h_next, c_next), res


def kernel(**inputs):
    (h_next, c_next), _ = run(inputs, trace=False)
    return (h_next, c_next)


# revision 30
# speedup vs baseline: 1.4074x; 1.4074x over previous
"""MetaLSTMCell Trainium2 kernel: 8 cores on a (batch x 2, hidden x 4) grid.

Core i handles batch rows bi*1024:(bi+1)*1024 (bi = i//4) and hidden columns
hi*256:(hi+1)*256 (hi = i%4) for all 4 gates.

v2 design (after trace analysis of the v1 baseline):
- ALL heavy preprocessing on host: gate permute to [i,f,o,g], hypernetwork
  fold M_* = d*_w @ z*_w, bias folds, per-core slicing/transposes, and bf16
  casts.  Device receives DMA-ready bf16 tiles (halves DMA bytes, removes
  every on-device cast that was stalling the PE).
- Inputs fully resident in SBUF (one big DMA each) so the MM stream has no
  per-tile DMA dependencies -> PE stays dense/warm (v1 ran at 1.2 GHz cold
  nearly all kernel: 241us throttled).
- Fused DVE ops: tensor_tensor_reduce produces y and the per-gate sums (and
  y^2 sums) in single passes; affine_mul_reduce applies (y-mu)*rs*ln_w in one
  op per gate.
- rsqrt(var+eps) via int bit-trick + 2 Newton steps on VectorE: the v1
  scalar-engine Sqrt forced ~2 activation-table reloads (~2.7us each) per
  batch tile (no table set holds both Sqrt and Sigmoid/Tanh).
- 4 grouped AllReduces (tiles 0-2 / 3-4 / 5-6 / 7, issued at bt 2/4/6/7)
  instead of 16 tiny ones: v1's collectives serialized at 6-36us each and
  gated a 107us tail; the final AR covers a single tile so it gates minimal
  tail work. (Measured: 2 bigger ARs regress — phase_b piles into the tail.)
- bf16 outputs (host upcasts); rel-err budget is 2e-2, measured 3.8e-3.

Measured on HW: 195us vs 289us baseline (1.48x), rel err 3.79e-3.
"""

import sys

sys.path.insert(0, "/opt/trn_rl_repo")

import numpy as np
import ml_dtypes
import concourse.bass as bass
import concourse.mybir as mybir
import concourse.tile as tile
from concourse.bass_utils import run_bass_kernel_spmd

B, IN, H, Z, G = 2048, 1024, 1024, 256, 4
NCORES = 8
BI_W, HI_W = 2, 4          # core grid: batch ways x hidden ways
BSH = B // BI_W            # 1024 batch rows per core
HSH = H // HI_W            # 256 hidden cols per core
HS = 128                   # h-subtile width
NHU = HSH // HS            # 2 h-subtiles per core
N = G * HS                 # 512: unit column width (4 gates x 128)
BT = 128                   # batch tile
NBT = BSH // BT            # 8 batch tiles per core
KC = IN // 128             # 8 K-chunks for the main GEMMs
ZC = Z // 128              # 2 K-chunks for the D GEMMs
PERM = (0, 1, 3, 2)        # gate order [i, f, o, g]
GRPS = ((0, 3), (3, 5), (5, 6), (6, 7), (7, 8))   # AllReduce tile groups
MAGIC = 0x5F375A86         # rsqrt seed constant
BF = ml_dtypes.bfloat16

dt = mybir.dt
AF = mybir.ActivationFunctionType
ALU = mybir.AluOpType
F32, BF16, I32 = dt.float32, dt.bfloat16, dt.int32


def fixup_multi_waits(nc):
    """This toolchain's walrus accepts at most ONE sync wait per instruction;
    Tile emits several. Hoist extras onto same-engine NOPs placed before."""
    for f in nc.m.functions:
        for blk in f.blocks:
            out = []
            changed = False
            for inst in blk.instructions:
                si = getattr(inst, "sync_info", None)
                waits = list(si.on_wait) if si is not None and si.on_wait else []
                if len(waits) > 1:
                    changed = True
                    for k, w in enumerate(waits[:-1]):
                        nop = mybir.InstNoOp(
                            name=f"{inst.name}-waitsplit{k}", ins=[], outs=[]
                        )
                        nop.engine = inst.engine
                        nop.sync_info = mybir.SyncInfo(on_wait=[w], on_update=[])
                        out.append(nop)
                    si.on_wait = [waits[-1]]
                out.append(inst)
            if changed:
                blk.instructions = out


def build(apply_fixup=True):
    nc = bass.Bass(trn_type="TRN2", num_devices=NCORES)
    P = 128

    def din(name, shape, dtype=BF16):
        return nc.dram_tensor(name, shape, dtype, kind="ExternalInput")

    xtt = din("xtt", [P, KC, NBT, BT])
    htt = din("htt", [P, KC, NBT, BT])
    mtt = din("mtt", [P, ZC, NBT, BT])
    ctt = din("ctt", [P, NBT, HSH])
    whbD = din("whbD", [P, NHU, KC, N])
    wxbD = din("wxbD", [P, NHU, KC, N])
    MhD = din("MhD", [P, NHU, ZC, N])
    MxD = din("MxD", [P, NHU, ZC, N])
    MbD = din("MbD", [P, NHU, ZC, N])
    rowsD = din("rowsD", [1, 3, NHU, N])
    lnwD = din("lnwD", [P, NHU, N])
    lnbD = din("lnbD", [P, NHU, N])
    hn = nc.dram_tensor("hn", [BSH, HSH], BF16, kind="ExternalOutput")
    cn = nc.dram_tensor("cn", [BSH, HSH], BF16, kind="ExternalOutput")

    quad_groups = [[0, 1, 2, 3], [4, 5, 6, 7]]

    with tile.TileContext(nc) as tc:
        with tc.tile_pool(name="wres", bufs=1) as wres, \
             tc.tile_pool(name="dram", bufs=1, space="DRAM") as dram, \
             tc.tile_pool(name="ev", bufs=3) as ev, \
             tc.tile_pool(name="mp", bufs=3) as mp, \
             tc.tile_pool(name="yp", bufs=12) as yp, \
             tc.tile_pool(name="gp", bufs=1) as gp, \
             tc.tile_pool(name="pb", bufs=3) as pb, \
             tc.tile_pool(name="ot", bufs=6) as ot, \
             tc.tile_pool(name="psd", bufs=4, space="PSUM") as psd, \
             tc.tile_pool(name="psw", bufs=4, space="PSUM") as psw:

            mom_in = dram.tile([BSH, 8], F32)
            mom_out = dram.tile([BSH, 8], F32)
            warm_in = dram.tile([1, 8], F32)
            warm_out = dram.tile([1, 8], F32)

            # warm-up collective: absorbs the CC entry barrier while the
            # weight DMAs stream in
            wz = wres.tile([1, 8], F32)
            nc.vector.memset(wz[:], 0.0)
            nc.sync.dma_start(warm_in[:], wz[:])
            nc.gpsimd.collective_compute(
                "AllReduce", ALU.add, replica_groups=quad_groups,
                ins=[warm_in[:]], outs=[warm_out[:]])

            # ---- persistent tiles / preamble DMAs (ordered so the D-GEMM
            # and WH-GEMM operands land first)
            Mh = wres.tile([P, NHU, ZC, N], BF16)
            Mx = wres.tile([P, NHU, ZC, N], BF16)
            Mb = wres.tile([P, NHU, ZC, N], BF16)
            mball = wres.tile([P, ZC, NBT, BT], BF16)
            b3 = wres.tile([P, 3, NHU, N], BF16)
            nc.vector.memset(b3[:], 0.0)
            e0 = wres.tile([P, P], BF16)
            nc.vector.memset(e0[:], 0.0)
            nc.vector.memset(e0[:1, :], 1.0)
            magic_t = wres.tile([P, 4, G], I32)
            nc.vector.memset(magic_t[:], MAGIC)
            one_t = wres.tile([P, 4, G], I32)
            nc.vector.memset(one_t[:], 1)

            nc.sync.dma_start(mball[:], mtt.ap())
            nc.sync.dma_start(Mh[:], MhD.ap())
            nc.sync.dma_start(Mx[:], MxD.ap())
            nc.sync.dma_start(Mb[:], MbD.ap())
            nc.sync.dma_start(b3[0:1], rowsD.ap())

            hball = wres.tile([P, KC, NBT, BT], BF16)
            nc.sync.dma_start(hball[:], htt.ap())
            whb = wres.tile([P, NHU, KC, N], BF16)
            for hu in range(NHU):
                nc.sync.dma_start(whb[:, hu], whbD.ap()[:, hu])
            xball = wres.tile([P, KC, NBT, BT], BF16)
            nc.sync.dma_start(xball[:], xtt.ap())
            wxb = wres.tile([P, NHU, KC, N], BF16)
            for hu in range(NHU):
                nc.sync.dma_start(wxb[:, hu], wxbD.ap()[:, hu])
            call = wres.tile([P, NBT, HSH], BF16)
            nc.sync.dma_start(call[:], ctt.ap())
            lnw_r = wres.tile([P, NHU, N], BF16)
            nc.sync.dma_start(lnw_r[:], lnwD.ap())
            lnb_r = wres.tile([P, NHU, N], BF16)
            nc.sync.dma_start(lnb_r[:], lnbD.ap())

            ytiles = {}
            obuf = {}
            rsg = {}
            nmg = {}

            def phase_a(bt):
                bs = slice(bt * BT, (bt + 1) * BT)
                mom = mp.tile([P, 8], F32, tag="mom")
                pm = mp.tile([P, 8], F32, tag="pm")
                obuf[bt] = (ot.tile([P, HSH], BF16, tag="cnb", name="cnb"),
                            ot.tile([P, HSH], BF16, tag="hnb", name="hnb"))
                for hu in range(NHU):
                    DH = psd.tile([P, N], F32, tag="psd")
                    DX = psd.tile([P, N], F32, tag="psd")
                    DB = psd.tile([P, N], F32, tag="psd")
                    for (D, MT, j) in ((DH, Mh, 0), (DX, Mx, 1), (DB, Mb, 2)):
                        for kc in range(ZC):
                            nc.tensor.matmul(D[:], mball[:, kc, bt],
                                             MT[:, hu, kc],
                                             start=(kc == 0), stop=False)
                        nc.tensor.matmul(D[:], e0[:], b3[:, j, hu],
                                         start=False, stop=True)
                    WH = psw.tile([P, N], F32, tag="psw")
                    for kc in range(KC):
                        nc.tensor.matmul(WH[:], hball[:, kc, bt],
                                         whb[:, hu, kc], start=(kc == 0),
                                         stop=(kc == KC - 1))
                    WX = psw.tile([P, N], F32, tag="psw")
                    for kc in range(KC):
                        nc.tensor.matmul(WX[:], xball[:, kc, bt],
                                         wxb[:, hu, kc], start=(kc == 0),
                                         stop=(kc == KC - 1))

                    dh_s = ev.tile([P, N], BF16, tag="dh_s")
                    nc.scalar.copy(dh_s[:], DH[:])
                    dx_s = ev.tile([P, N], BF16, tag="dx_s")
                    nc.scalar.copy(dx_s[:], DX[:])
                    db_s = ev.tile([P, N], BF16, tag="db_s")
                    nc.scalar.copy(db_s[:], DB[:])
                    wx_s = ev.tile([P, N], BF16, tag="wx_s")
                    nc.scalar.copy(wx_s[:], WX[:])

                    y1 = ev.tile([P, N], BF16, tag="y1")
                    nc.vector.tensor_mul(y1[:], WH[:], dh_s[:])
                    y2 = ev.tile([P, N], BF16, tag="y2")
                    nc.gpsimd.tensor_mul(y2[:], wx_s[:], dx_s[:])
                    y12 = ev.tile([P, N], BF16, tag="y12")
                    nc.vector.tensor_add(y12[:], y1[:], y2[:])

                    y = yp.tile([P, N], BF16, tag="y")
                    ysq = ev.tile([P, N], BF16, tag="ysq")
                    acc = pm if hu == 0 else mom
                    for g in range(G):
                        gs = slice(g * HS, (g + 1) * HS)
                        nc.vector.scalar_tensor_tensor(
                            y[:, gs], y12[:, gs], 1.0, db_s[:, gs],
                            ALU.mult, ALU.add, accum_out=acc[:, g:g + 1])
                    for g in range(G):
                        gs = slice(g * HS, (g + 1) * HS)
                        nc.vector.scalar_tensor_tensor(
                            ysq[:, gs], y[:, gs], 1.0, y[:, gs],
                            ALU.mult, ALU.mult, accum_out=acc[:, 4 + g:5 + g])
                    ytiles[(bt, hu)] = y
                nc.vector.tensor_add(mom[:], mom[:], pm[:])
                nc.sync.dma_start(mom_in[bs, :], mom[:])

            def ar(grp):
                t0, t1 = GRPS[grp]
                rows = slice(t0 * BT, t1 * BT)
                nc.gpsimd.collective_compute(
                    "AllReduce", ALU.add, replica_groups=quad_groups,
                    ins=[mom_in[rows, :]], outs=[mom_out[rows, :]])

            def prep(grp):
                t0, t1 = GRPS[grp]
                nt = t1 - t0
                gm = gp.tile([P, nt, 8], F32, tag=f"gm{grp}")
                nc.sync.dma_start(
                    gm[:], mom_out[t0 * BT:t1 * BT, :]
                    .rearrange("(t p) c -> p t c", p=P))
                scl = gp.tile([P, nt, 8], F32, tag=f"scl{grp}")
                nc.vector.tensor_scalar_mul(scl[:], gm[:], 1.0 / H)
                mu = scl[:, :, 0:4]
                msq = scl[:, :, 4:8]
                musq = gp.tile([P, nt, G], F32, tag=f"musq{grp}")
                nc.vector.tensor_mul(musq[:], mu, mu)
                veps = gp.tile([P, nt, G], F32, tag=f"veps{grp}")
                nc.vector.scalar_tensor_tensor(
                    veps[:], musq[:], -1.0, msq, ALU.mult, ALU.add)
                nc.vector.tensor_scalar_add(veps[:], veps[:], 1e-5)
                sh = gp.tile([P, nt, G], I32, tag=f"sh{grp}")
                nc.vector.tensor_tensor(sh[:], veps[:].bitcast(I32),
                                        one_t[:, :nt], ALU.logical_shift_right)
                x = gp.tile([P, nt, G], F32, tag=f"x{grp}")
                nc.vector.tensor_tensor(x[:].bitcast(I32), magic_t[:, :nt],
                                        sh[:], ALU.subtract)
                a = gp.tile([P, nt, G], F32, tag=f"a{grp}")
                b2 = gp.tile([P, nt, G], F32, tag=f"b2{grp}")
                for _ in range(1):
                    nc.vector.tensor_mul(a[:], x[:], x[:])
                    nc.vector.tensor_mul(b2[:], a[:], veps[:])
                    nc.vector.tensor_scalar(b2[:], b2[:], -0.5, 1.5,
                                            op0=ALU.mult, op1=ALU.add)
                    nc.vector.tensor_mul(x[:], x[:], b2[:])
                nmt = gp.tile([P, nt, G], F32, tag=f"nmt{grp}")
                nc.vector.scalar_tensor_tensor(
                    nmt[:], mu, -1.0, x[:], ALU.mult, ALU.mult)
                rsg[grp] = x
                nmg[grp] = nmt

            def phase_b(bt):
                grp = next(g for g, (t0, t1) in enumerate(GRPS)
                           if t0 <= bt < t1)
                gi = bt - GRPS[grp][0]
                rs = rsg[grp]
                nm = nmg[grp]
                cn_bt, hn_bt = obuf.pop(bt)
                bs = slice(bt * BT, (bt + 1) * BT)
                for hu in range(NHU):
                    y = ytiles.pop((bt, hu))
                    u = pb.tile([P, N], BF16, tag="u")
                    for g in range(G):
                        gs = slice(g * HS, (g + 1) * HS)
                        nc.vector.tensor_scalar(
                            u[:, gs], y[:, gs], rs[:, gi, g:g + 1],
                            nm[:, gi, g:g + 1], op0=ALU.mult, op1=ALU.add)
                    vv = pb.tile([P, N], BF16, tag="vv")
                    nc.vector.tensor_mul(vv[:], u[:], lnw_r[:, hu])
                    vv2 = pb.tile([P, N], BF16, tag="vv2")
                    nc.vector.tensor_add(vv2[:], vv[:], lnb_r[:, hu])
                    gt = pb.tile([P, N], BF16, tag="gt")
                    nc.scalar.activation(gt[:, 0:3 * HS], vv2[:, 0:3 * HS],
                                         AF.Sigmoid)
                    nc.scalar.activation(gt[:, 3 * HS:N], vv2[:, 3 * HS:N],
                                         AF.Tanh)
                    hs_cols = slice(hu * HS, (hu + 1) * HS)
                    cs = call[:, bt, hs_cols]
                    sfc = pb.tile([P, HS], BF16, tag="sfc")
                    nc.vector.tensor_mul(sfc[:], gt[:, HS:2 * HS], cs)
                    sit = pb.tile([P, HS], BF16, tag="sit")
                    nc.vector.tensor_mul(sit[:], gt[:, 0:HS], gt[:, 3 * HS:N])
                    nc.gpsimd.tensor_add(cn_bt[:, hs_cols], sfc[:], sit[:])
                    tc_t = pb.tile([P, HS], BF16, tag="tc")
                    nc.scalar.activation(tc_t[:], cn_bt[:, hs_cols], AF.Tanh)
                    nc.gpsimd.tensor_mul(hn_bt[:, hs_cols],
                                         gt[:, 2 * HS:3 * HS], tc_t[:])
                nc.sync.dma_start(cn[bs, :], cn_bt[:])
                nc.sync.dma_start(hn[bs, :], hn_bt[:])

            # ---- main schedule: ARs at bt 2/4/5/6/7. Each late AR covers
            # one tile: it starts as soon as that tile's moments land, and
            # each collective resyncs the quad so skew cannot accumulate
            # into the final one.
            for bt in range(NBT):
                phase_a(bt)
                if bt == 2:
                    ar(0)
                if bt == 4:
                    ar(1)
                if bt == 5:
                    ar(2)
                if bt == 6:
                    ar(3)
                if bt == 7:
                    ar(4)
                if bt == 4:
                    prep(0)
                if bt == 6:
                    prep(1)
                if bt == 7:
                    prep(2)
                if bt >= 4 and bt <= 6:
                    phase_b(bt - 4)
            phase_b(3)
            phase_b(4)
            prep(3)
            phase_b(5)
            phase_b(6)
            prep(4)
            phase_b(7)

    if apply_fixup:
        fixup_multi_waits(nc)
    return nc


_nc = None


def _get_nc():
    global _nc
    if _nc is None:
        _nc = build()
    return _nc


def make_in_maps(src_x, h, c, src_meta, zh_w, zh_b, zx_w, zx_b, zb_w,
                 dh_w, dx_w, db_w, db_b, w_h, w_x, ln_w, ln_b):
    f32 = np.float32
    asc = np.ascontiguousarray
    perm = list(PERM)
    w_h = w_h[perm]
    w_x = w_x[perm]
    dh_w = dh_w[perm]
    dx_w = dx_w[perm]
    db_w = db_w[perm]
    db_b = db_b[perm]
    ln_w = ln_w[perm]
    ln_b = ln_b[perm]
    zh_w3 = zh_w.reshape(G, Z, Z)[perm]
    zx_w3 = zx_w.reshape(G, Z, Z)[perm]
    zb_w3 = zb_w.reshape(G, Z, Z)[perm]
    zh_b2 = zh_b.reshape(G, Z)[perm]
    zx_b2 = zx_b.reshape(G, Z)[perm]

    # hypernetwork fold: M*[g,h,z'] = sum_z d*_w[g,h,z] * z*_w[g,z,z']
    Mh_full = np.matmul(dh_w, zh_w3)          # [G, H, Z]
    Mx_full = np.matmul(dx_w, zx_w3)
    Mb_full = np.matmul(db_w, zb_w3)

    xT = asc(src_x.T.astype(f32, copy=False))  # [IN, B]
    hT = asc(h.T.astype(f32, copy=False))
    mT = asc(src_meta.T.astype(f32, copy=False))

    def act_tiles(aT, brows, kchunks):
        # [K, BSH] -> [128p, kc, bt, bb] bf16
        sl = aT[:, brows]
        return asc(sl.reshape(kchunks, 128, NBT, BT)
                   .transpose(1, 0, 2, 3).astype(BF))

    def per_hu_w(w):
        # [G, HSH, IN] slice -> [128p, NHU, KC, N] bf16
        out = np.empty((NHU, IN, N), f32)
        for hu in range(NHU):
            blk = w[:, hu * HS:(hu + 1) * HS, :]   # [G, HS, IN]
            out[hu] = blk.transpose(2, 0, 1).reshape(IN, N)
        return asc(out.reshape(NHU, KC, 128, N)
                   .transpose(2, 0, 1, 3).astype(BF))

    def per_hu_m(Mfull_sl):
        # [G, HSH, Z] slice -> [128p, NHU, ZC, N] bf16
        out = np.empty((NHU, Z, N), f32)
        for hu in range(NHU):
            blk = Mfull_sl[:, hu * HS:(hu + 1) * HS, :]  # [G, HS, Z]
            out[hu] = blk.transpose(2, 0, 1).reshape(Z, N)
        return asc(out.reshape(NHU, ZC, 128, N)
                   .transpose(2, 0, 1, 3).astype(BF))

    def per_hu_row(v):
        # v: [G, HSH] -> [NHU, N] with [hu][g*HS+hh]
        return (v.reshape(G, NHU, HS).transpose(1, 0, 2)
                .reshape(NHU, N).astype(f32))

    in_maps = []
    for ci in range(NCORES):
        bi, hi = ci // HI_W, ci % HI_W
        brows = slice(bi * BSH, (bi + 1) * BSH)
        hcols = slice(hi * HSH, (hi + 1) * HSH)

        bdh_c = np.einsum("gz,ghz->gh", zh_b2, dh_w[:, hcols, :])
        bdx_c = np.einsum("gz,ghz->gh", zx_b2, dx_w[:, hcols, :])
        rows3 = np.stack([per_hu_row(bdh_c), per_hu_row(bdx_c),
                          per_hu_row(db_b[:, hcols])])  # [3, NHU, N]

        cb = c[brows, hcols].reshape(NBT, 128, HSH).transpose(1, 0, 2)

        in_maps.append({
            "xtt": act_tiles(xT, brows, KC),
            "htt": act_tiles(hT, brows, KC),
            "mtt": act_tiles(mT, brows, ZC),
            "ctt": asc(cb.astype(BF)),
            "whbD": per_hu_w(w_h[:, hcols, :]),
            "wxbD": per_hu_w(w_x[:, hcols, :]),
            "MhD": per_hu_m(Mh_full[:, hcols, :]),
            "MxD": per_hu_m(Mx_full[:, hcols, :]),
            "MbD": per_hu_m(Mb_full[:, hcols, :]),
            "rowsD": asc(rows3[None].astype(BF)),
            "lnwD": asc(np.broadcast_to(per_hu_row(ln_w[:, hcols])[None],
                                        (128, NHU, N)).astype(BF)),
            "lnbD": asc(np.broadcast_to(per_hu_row(ln_b[:, hcols])[None],
                                        (128, NHU, N)).astype(BF)),
        })
    return in_maps


def run(inputs, trace=False):
    nc = _get_nc()
    in_maps = make_in_maps(**inputs)
    res = run_bass_kernel_spmd(nc, in_maps, core_ids=list(range(NCORES)),
                               trace=trace)
    h_next = np.empty((B, H), np.float32)
    c_next = np.empty((B, H), np.float32)
    for ci in range(NCORES):
        bi, hi = ci // HI_W, ci % HI_W
        brows = slice(bi * BSH, (bi + 1) * BSH)
        hcols = slice(hi * HSH, (hi + 1) * HSH)
        h_next[brows, hcols] = np.asarray(res.results[ci]["hn"]).astype(
            np.float32)
        c_next[brows, hcols] = np.asarray(res.results[ci]["cn"]).astype(
            np.float32)
    return (h_next, c_next), res


def kernel(**inputs):
    (h_next, c_next), _ = run(inputs, trace=False)
    return (h_next, c_next)


# revision 32
# speedup vs baseline: 1.4538x; 1.0330x over previous
"""MetaLSTMCell Trainium2 kernel: 8 cores on a (batch x 2, hidden x 4) grid.

Core i handles batch rows bi*1024:(bi+1)*1024 (bi = i//4) and hidden columns
hi*256:(hi+1)*256 (hi = i%4) for all 4 gates.

v2 design (after trace analysis of the v1 baseline):
- ALL heavy preprocessing on host: gate permute to [i,f,o,g], hypernetwork
  fold M_* = d*_w @ z*_w, bias folds, per-core slicing/transposes, and bf16
  casts.  Device receives DMA-ready bf16 tiles (halves DMA bytes, removes
  every on-device cast that was stalling the PE).
- Inputs fully resident in SBUF (one big DMA each) so the MM stream has no
  per-tile DMA dependencies -> PE stays dense/warm (v1 ran at 1.2 GHz cold
  nearly all kernel: 241us throttled).
- Fused DVE ops: tensor_tensor_reduce produces y and the per-gate sums (and
  y^2 sums) in single passes; affine_mul_reduce applies (y-mu)*rs*ln_w in one
  op per gate.
- rsqrt(var+eps) via int bit-trick + 2 Newton steps on VectorE: the v1
  scalar-engine Sqrt forced ~2 activation-table reloads (~2.7us each) per
  batch tile (no table set holds both Sqrt and Sigmoid/Tanh).
- 4 grouped AllReduces (tiles 0-2 / 3-4 / 5-6 / 7, issued at bt 2/4/6/7)
  instead of 16 tiny ones: v1's collectives serialized at 6-36us each and
  gated a 107us tail; the final AR covers a single tile so it gates minimal
  tail work. (Measured: 2 bigger ARs regress — phase_b piles into the tail.)
- bf16 outputs (host upcasts); rel-err budget is 2e-2, measured 3.8e-3.

Measured on HW: 195us vs 289us baseline (1.48x), rel err 3.79e-3.
"""

import sys

sys.path.insert(0, "/opt/trn_rl_repo")

import numpy as np
import ml_dtypes
import concourse.bass as bass
import concourse.mybir as mybir
import concourse.tile as tile
from concourse.bass_utils import run_bass_kernel_spmd

B, IN, H, Z, G = 2048, 1024, 1024, 256, 4
NCORES = 8
BI_W, HI_W = 2, 4          # core grid: batch ways x hidden ways
BSH = B // BI_W            # 1024 batch rows per core
HSH = H // HI_W            # 256 hidden cols per core
HS = 128                   # h-subtile width
NHU = HSH // HS            # 2 h-subtiles per core
N = G * HS                 # 512: unit column width (4 gates x 128)
BT = 128                   # batch tile
NBT = BSH // BT            # 8 batch tiles per core
KC = IN // 128             # 8 K-chunks for the main GEMMs
ZC = Z // 128              # 2 K-chunks for the D GEMMs
PERM = (0, 1, 3, 2)        # gate order [i, f, o, g]
GRPS = ((0, 2), (2, 4), (4, 6), (6, 7), (7, 8))   # AllReduce tile groups
MAGIC = 0x5F375A86         # rsqrt seed constant
BF = ml_dtypes.bfloat16

dt = mybir.dt
AF = mybir.ActivationFunctionType
ALU = mybir.AluOpType
F32, BF16, I32 = dt.float32, dt.bfloat16, dt.int32


def fixup_multi_waits(nc):
    """This toolchain's walrus accepts at most ONE sync wait per instruction;
    Tile emits several. Hoist extras onto same-engine NOPs placed before."""
    for f in nc.m.functions:
        for blk in f.blocks:
            out = []
            changed = False
            for inst in blk.instructions:
                si = getattr(inst, "sync_info", None)
                waits = list(si.on_wait) if si is not None and si.on_wait else []
                if len(waits) > 1:
                    changed = True
                    for k, w in enumerate(waits[:-1]):
                        nop = mybir.InstNoOp(
                            name=f"{inst.name}-waitsplit{k}", ins=[], outs=[]
                        )
                        nop.engine = inst.engine
                        nop.sync_info = mybir.SyncInfo(on_wait=[w], on_update=[])
                        out.append(nop)
                    si.on_wait = [waits[-1]]
                out.append(inst)
            if changed:
                blk.instructions = out


def build(apply_fixup=True):
    nc = bass.Bass(trn_type="TRN2", num_devices=NCORES)
    P = 128

    def din(name, shape, dtype=BF16):
        return nc.dram_tensor(name, shape, dtype, kind="ExternalInput")

    xtt = din("xtt", [P, KC, NBT, BT])
    htt = din("htt", [P, KC, NBT, BT])
    mtt = din("mtt", [P, ZC, NBT, BT])
    ctt = din("ctt", [P, NBT, HSH])
    whbD = din("whbD", [P, NHU, KC, N])
    wxbD = din("wxbD", [P, NHU, KC, N])
    MhD = din("MhD", [P, NHU, ZC, N])
    MxD = din("MxD", [P, NHU, ZC, N])
    MbD = din("MbD", [P, NHU, ZC, N])
    rowsD = din("rowsD", [1, 3, NHU, N])
    lnwD = din("lnwD", [P, NHU, N])
    lnbD = din("lnbD", [P, NHU, N])
    hn = nc.dram_tensor("hn", [BSH, HSH], BF16, kind="ExternalOutput")
    cn = nc.dram_tensor("cn", [BSH, HSH], BF16, kind="ExternalOutput")

    quad_groups = [[0, 1, 2, 3], [4, 5, 6, 7]]

    with tile.TileContext(nc) as tc:
        with tc.tile_pool(name="wres", bufs=1) as wres, \
             tc.tile_pool(name="dram", bufs=1, space="DRAM") as dram, \
             tc.tile_pool(name="ev", bufs=3) as ev, \
             tc.tile_pool(name="mp", bufs=3) as mp, \
             tc.tile_pool(name="yp", bufs=12) as yp, \
             tc.tile_pool(name="gp", bufs=1) as gp, \
             tc.tile_pool(name="pb", bufs=3) as pb, \
             tc.tile_pool(name="ot", bufs=6) as ot, \
             tc.tile_pool(name="psd", bufs=4, space="PSUM") as psd, \
             tc.tile_pool(name="psw", bufs=4, space="PSUM") as psw:

            mom_in = dram.tile([BSH, 8], F32)
            mom_out = dram.tile([BSH, 8], F32)
            warm_in = dram.tile([1, 8], F32)
            warm_out = dram.tile([1, 8], F32)

            # warm-up collective: absorbs the CC entry barrier while the
            # weight DMAs stream in
            wz = wres.tile([1, 8], F32)
            nc.vector.memset(wz[:], 0.0)
            nc.sync.dma_start(warm_in[:], wz[:])
            nc.gpsimd.collective_compute(
                "AllReduce", ALU.add, replica_groups=quad_groups,
                ins=[warm_in[:]], outs=[warm_out[:]])

            # ---- persistent tiles / preamble DMAs (ordered so the D-GEMM
            # and WH-GEMM operands land first)
            Mh = wres.tile([P, NHU, ZC, N], BF16)
            Mx = wres.tile([P, NHU, ZC, N], BF16)
            Mb = wres.tile([P, NHU, ZC, N], BF16)
            mball = wres.tile([P, ZC, NBT, BT], BF16)
            b3 = wres.tile([P, 3, NHU, N], BF16)
            nc.vector.memset(b3[:], 0.0)
            e0 = wres.tile([P, P], BF16)
            nc.vector.memset(e0[:], 0.0)
            nc.vector.memset(e0[:1, :], 1.0)
            magic_t = wres.tile([P, 4, G], I32)
            nc.vector.memset(magic_t[:], MAGIC)
            one_t = wres.tile([P, 4, G], I32)
            nc.vector.memset(one_t[:], 1)

            nc.sync.dma_start(mball[:], mtt.ap())
            nc.sync.dma_start(Mh[:], MhD.ap())
            nc.sync.dma_start(Mx[:], MxD.ap())
            nc.sync.dma_start(Mb[:], MbD.ap())
            nc.sync.dma_start(b3[0:1], rowsD.ap())

            hball = wres.tile([P, KC, NBT, BT], BF16)
            nc.sync.dma_start(hball[:], htt.ap())
            whb = wres.tile([P, NHU, KC, N], BF16)
            for hu in range(NHU):
                nc.sync.dma_start(whb[:, hu], whbD.ap()[:, hu])
            xball = wres.tile([P, KC, NBT, BT], BF16)
            nc.sync.dma_start(xball[:], xtt.ap())
            wxb = wres.tile([P, NHU, KC, N], BF16)
            for hu in range(NHU):
                nc.sync.dma_start(wxb[:, hu], wxbD.ap()[:, hu])
            call = wres.tile([P, NBT, HSH], BF16)
            nc.sync.dma_start(call[:], ctt.ap())
            lnw_r = wres.tile([P, NHU, N], BF16)
            nc.sync.dma_start(lnw_r[:], lnwD.ap())
            lnb_r = wres.tile([P, NHU, N], BF16)
            nc.sync.dma_start(lnb_r[:], lnbD.ap())

            ytiles = {}
            obuf = {}
            rsg = {}
            nmg = {}

            def phase_a(bt):
                bs = slice(bt * BT, (bt + 1) * BT)
                mom = mp.tile([P, 8], F32, tag="mom")
                pm = mp.tile([P, 8], F32, tag="pm")
                obuf[bt] = (ot.tile([P, HSH], BF16, tag="cnb", name="cnb"),
                            ot.tile([P, HSH], BF16, tag="hnb", name="hnb"))
                for hu in range(NHU):
                    DH = psd.tile([P, N], F32, tag="psd")
                    DX = psd.tile([P, N], F32, tag="psd")
                    DB = psd.tile([P, N], F32, tag="psd")
                    for (D, MT, j) in ((DH, Mh, 0), (DX, Mx, 1), (DB, Mb, 2)):
                        for kc in range(ZC):
                            nc.tensor.matmul(D[:], mball[:, kc, bt],
                                             MT[:, hu, kc],
                                             start=(kc == 0), stop=False)
                        nc.tensor.matmul(D[:], e0[:], b3[:, j, hu],
                                         start=False, stop=True)
                    WH = psw.tile([P, N], F32, tag="psw")
                    for kc in range(KC):
                        nc.tensor.matmul(WH[:], hball[:, kc, bt],
                                         whb[:, hu, kc], start=(kc == 0),
                                         stop=(kc == KC - 1))
                    WX = psw.tile([P, N], F32, tag="psw")
                    for kc in range(KC):
                        nc.tensor.matmul(WX[:], xball[:, kc, bt],
                                         wxb[:, hu, kc], start=(kc == 0),
                                         stop=(kc == KC - 1))

                    dh_s = ev.tile([P, N], BF16, tag="dh_s")
                    nc.scalar.copy(dh_s[:], DH[:])
                    dx_s = ev.tile([P, N], BF16, tag="dx_s")
                    nc.scalar.copy(dx_s[:], DX[:])
                    db_s = ev.tile([P, N], BF16, tag="db_s")
                    nc.scalar.copy(db_s[:], DB[:])
                    wx_s = ev.tile([P, N], BF16, tag="wx_s")
                    nc.scalar.copy(wx_s[:], WX[:])

                    y1 = ev.tile([P, N], BF16, tag="y1")
                    nc.vector.tensor_mul(y1[:], WH[:], dh_s[:])
                    y2 = ev.tile([P, N], BF16, tag="y2")
                    nc.gpsimd.tensor_mul(y2[:], wx_s[:], dx_s[:])
                    y12 = ev.tile([P, N], BF16, tag="y12")
                    nc.vector.tensor_add(y12[:], y1[:], y2[:])

                    y = yp.tile([P, N], BF16, tag="y")
                    ysq = ev.tile([P, N], BF16, tag="ysq")
                    acc = pm if hu == 0 else mom
                    for g in range(G):
                        gs = slice(g * HS, (g + 1) * HS)
                        nc.vector.scalar_tensor_tensor(
                            y[:, gs], y12[:, gs], 1.0, db_s[:, gs],
                            ALU.mult, ALU.add, accum_out=acc[:, g:g + 1])
                    for g in range(G):
                        gs = slice(g * HS, (g + 1) * HS)
                        nc.vector.scalar_tensor_tensor(
                            ysq[:, gs], y[:, gs], 1.0, y[:, gs],
                            ALU.mult, ALU.mult, accum_out=acc[:, 4 + g:5 + g])
                    ytiles[(bt, hu)] = y
                nc.vector.tensor_add(mom[:], mom[:], pm[:])
                nc.sync.dma_start(mom_in[bs, :], mom[:])

            def ar(grp):
                t0, t1 = GRPS[grp]
                rows = slice(t0 * BT, t1 * BT)
                nc.gpsimd.collective_compute(
                    "AllReduce", ALU.add, replica_groups=quad_groups,
                    ins=[mom_in[rows, :]], outs=[mom_out[rows, :]])

            def prep(grp):
                t0, t1 = GRPS[grp]
                nt = t1 - t0
                gm = gp.tile([P, nt, 8], F32, tag=f"gm{grp}")
                nc.sync.dma_start(
                    gm[:], mom_out[t0 * BT:t1 * BT, :]
                    .rearrange("(t p) c -> p t c", p=P))
                scl = gp.tile([P, nt, 8], F32, tag=f"scl{grp}")
                nc.vector.tensor_scalar_mul(scl[:], gm[:], 1.0 / H)
                mu = scl[:, :, 0:4]
                msq = scl[:, :, 4:8]
                musq = gp.tile([P, nt, G], F32, tag=f"musq{grp}")
                nc.vector.tensor_mul(musq[:], mu, mu)
                veps = gp.tile([P, nt, G], F32, tag=f"veps{grp}")
                nc.vector.scalar_tensor_tensor(
                    veps[:], musq[:], -1.0, msq, ALU.mult, ALU.add)
                nc.vector.tensor_scalar_add(veps[:], veps[:], 1e-5)
                sh = gp.tile([P, nt, G], I32, tag=f"sh{grp}")
                nc.vector.tensor_tensor(sh[:], veps[:].bitcast(I32),
                                        one_t[:, :nt], ALU.logical_shift_right)
                x = gp.tile([P, nt, G], F32, tag=f"x{grp}")
                nc.vector.tensor_tensor(x[:].bitcast(I32), magic_t[:, :nt],
                                        sh[:], ALU.subtract)
                a = gp.tile([P, nt, G], F32, tag=f"a{grp}")
                b2 = gp.tile([P, nt, G], F32, tag=f"b2{grp}")
                for _ in range(1):
                    nc.vector.tensor_mul(a[:], x[:], x[:])
                    nc.vector.tensor_mul(b2[:], a[:], veps[:])
                    nc.vector.tensor_scalar(b2[:], b2[:], -0.5, 1.5,
                                            op0=ALU.mult, op1=ALU.add)
                    nc.vector.tensor_mul(x[:], x[:], b2[:])
                nmt = gp.tile([P, nt, G], F32, tag=f"nmt{grp}")
                nc.vector.scalar_tensor_tensor(
                    nmt[:], mu, -1.0, x[:], ALU.mult, ALU.mult)
                rsg[grp] = x
                nmg[grp] = nmt

            def phase_b(bt):
                grp = next(g for g, (t0, t1) in enumerate(GRPS)
                           if t0 <= bt < t1)
                gi = bt - GRPS[grp][0]
                rs = rsg[grp]
                nm = nmg[grp]
                cn_bt, hn_bt = obuf.pop(bt)
                bs = slice(bt * BT, (bt + 1) * BT)
                for hu in range(NHU):
                    y = ytiles.pop((bt, hu))
                    u = pb.tile([P, N], BF16, tag="u")
                    for g in range(G):
                        gs = slice(g * HS, (g + 1) * HS)
                        nc.vector.tensor_scalar(
                            u[:, gs], y[:, gs], rs[:, gi, g:g + 1],
                            nm[:, gi, g:g + 1], op0=ALU.mult, op1=ALU.add)
                    vv = pb.tile([P, N], BF16, tag="vv")
                    nc.vector.tensor_mul(vv[:], u[:], lnw_r[:, hu])
                    vv2 = pb.tile([P, N], BF16, tag="vv2")
                    nc.vector.tensor_add(vv2[:], vv[:], lnb_r[:, hu])
                    gt = pb.tile([P, N], BF16, tag="gt")
                    nc.scalar.activation(gt[:, 0:3 * HS], vv2[:, 0:3 * HS],
                                         AF.Sigmoid)
                    nc.scalar.activation(gt[:, 3 * HS:N], vv2[:, 3 * HS:N],
                                         AF.Tanh)
                    hs_cols = slice(hu * HS, (hu + 1) * HS)
                    cs = call[:, bt, hs_cols]
                    sfc = pb.tile([P, HS], BF16, tag="sfc")
                    nc.vector.tensor_mul(sfc[:], gt[:, HS:2 * HS], cs)
                    sit = pb.tile([P, HS], BF16, tag="sit")
                    nc.vector.tensor_mul(sit[:], gt[:, 0:HS], gt[:, 3 * HS:N])
                    nc.gpsimd.tensor_add(cn_bt[:, hs_cols], sfc[:], sit[:])
                    tc_t = pb.tile([P, HS], BF16, tag="tc")
                    nc.scalar.activation(tc_t[:], cn_bt[:, hs_cols], AF.Tanh)
                    nc.gpsimd.tensor_mul(hn_bt[:, hs_cols],
                                         gt[:, 2 * HS:3 * HS], tc_t[:])
                nc.sync.dma_start(cn[bs, :], cn_bt[:])
                nc.sync.dma_start(hn[bs, :], hn_bt[:])

            # ---- main schedule: AR pairs at bt 1/3/5, singles at 6/7.
            # phase_b starts at bt3 so 5 of 8 tiles drain during the MM
            # phase; only pb(7) is gated by the final collective.
            for bt in range(NBT):
                phase_a(bt)
                if bt == 1:
                    ar(0)
                if bt == 3:
                    ar(1)
                if bt == 5:
                    ar(2)
                if bt == 6:
                    ar(3)
                if bt == 7:
                    ar(4)
                if bt == 3:
                    prep(0)
                if bt == 5:
                    prep(1)
                if bt == 7:
                    prep(2)
                if bt >= 3:
                    phase_b(bt - 3)
            phase_b(5)
            prep(3)
            phase_b(6)
            prep(4)
            phase_b(7)

    if apply_fixup:
        fixup_multi_waits(nc)
    return nc


_nc = None


def _get_nc():
    global _nc
    if _nc is None:
        _nc = build()
    return _nc


def make_in_maps(src_x, h, c, src_meta, zh_w, zh_b, zx_w, zx_b, zb_w,
                 dh_w, dx_w, db_w, db_b, w_h, w_x, ln_w, ln_b):
    f32 = np.float32
    asc = np.ascontiguousarray
    perm = list(PERM)
    w_h = w_h[perm]
    w_x = w_x[perm]
    dh_w = dh_w[perm]
    dx_w = dx_w[perm]
    db_w = db_w[perm]
    db_b = db_b[perm]
    ln_w = ln_w[perm]
    ln_b = ln_b[perm]
    zh_w3 = zh_w.reshape(G, Z, Z)[perm]
    zx_w3 = zx_w.reshape(G, Z, Z)[perm]
    zb_w3 = zb_w.reshape(G, Z, Z)[perm]
    zh_b2 = zh_b.reshape(G, Z)[perm]
    zx_b2 = zx_b.reshape(G, Z)[perm]

    # hypernetwork fold: M*[g,h,z'] = sum_z d*_w[g,h,z] * z*_w[g,z,z']
    Mh_full = np.matmul(dh_w, zh_w3)          # [G, H, Z]
    Mx_full = np.matmul(dx_w, zx_w3)
    Mb_full = np.matmul(db_w, zb_w3)

    xT = asc(src_x.T.astype(f32, copy=False))  # [IN, B]
    hT = asc(h.T.astype(f32, copy=False))
    mT = asc(src_meta.T.astype(f32, copy=False))

    def act_tiles(aT, brows, kchunks):
        # [K, BSH] -> [128p, kc, bt, bb] bf16
        sl = aT[:, brows]
        return asc(sl.reshape(kchunks, 128, NBT, BT)
                   .transpose(1, 0, 2, 3).astype(BF))

    def per_hu_w(w):
        # [G, HSH, IN] slice -> [128p, NHU, KC, N] bf16
        out = np.empty((NHU, IN, N), f32)
        for hu in range(NHU):
            blk = w[:, hu * HS:(hu + 1) * HS, :]   # [G, HS, IN]
            out[hu] = blk.transpose(2, 0, 1).reshape(IN, N)
        return asc(out.reshape(NHU, KC, 128, N)
                   .transpose(2, 0, 1, 3).astype(BF))

    def per_hu_m(Mfull_sl):
        # [G, HSH, Z] slice -> [128p, NHU, ZC, N] bf16
        out = np.empty((NHU, Z, N), f32)
        for hu in range(NHU):
            blk = Mfull_sl[:, hu * HS:(hu + 1) * HS, :]  # [G, HS, Z]
            out[hu] = blk.transpose(2, 0, 1).reshape(Z, N)
        return asc(out.reshape(NHU, ZC, 128, N)
                   .transpose(2, 0, 1, 3).astype(BF))

    def per_hu_row(v):
        # v: [G, HSH] -> [NHU, N] with [hu][g*HS+hh]
        return (v.reshape(G, NHU, HS).transpose(1, 0, 2)
                .reshape(NHU, N).astype(f32))

    in_maps = []
    for ci in range(NCORES):
        bi, hi = ci // HI_W, ci % HI_W
        brows = slice(bi * BSH, (bi + 1) * BSH)
        hcols = slice(hi * HSH, (hi + 1) * HSH)

        bdh_c = np.einsum("gz,ghz->gh", zh_b2, dh_w[:, hcols, :])
        bdx_c = np.einsum("gz,ghz->gh", zx_b2, dx_w[:, hcols, :])
        rows3 = np.stack([per_hu_row(bdh_c), per_hu_row(bdx_c),
                          per_hu_row(db_b[:, hcols])])  # [3, NHU, N]

        cb = c[brows, hcols].reshape(NBT, 128, HSH).transpose(1, 0, 2)

        in_maps.append({
            "xtt": act_tiles(xT, brows, KC),
            "htt": act_tiles(hT, brows, KC),
            "mtt": act_tiles(mT, brows, ZC),
            "ctt": asc(cb.astype(BF)),
            "whbD": per_hu_w(w_h[:, hcols, :]),
            "wxbD": per_hu_w(w_x[:, hcols, :]),
            "MhD": per_hu_m(Mh_full[:, hcols, :]),
            "MxD": per_hu_m(Mx_full[:, hcols, :]),
            "MbD": per_hu_m(Mb_full[:, hcols, :]),
            "rowsD": asc(rows3[None].astype(BF)),
            "lnwD": asc(np.broadcast_to(per_hu_row(ln_w[:, hcols])[None],
                                        (128, NHU, N)).astype(BF)),
            "lnbD": asc(np.broadcast_to(per_hu_row(ln_b[:, hcols])[None],
                                        (128, NHU, N)).astype(BF)),
        })
    return in_maps


def run(inputs, trace=False):
    nc = _get_nc()
    in_maps = make_in_maps(**inputs)
    res = run_bass_kernel_spmd(nc, in_maps, core_ids=list(range(NCORES)),
                               trace=trace)
    h_next = np.empty((B, H), np.float32)
    c_next = np.empty((B, H), np.float32)
    for ci in range(NCORES):
        bi, hi = ci // HI_W, ci % HI_W
        brows = slice(bi * BSH, (bi + 1) * BSH)
        hcols = slice(hi * HSH, (hi + 1) * HSH)
        h_next[brows, hcols] = np.asarray(res.results[ci]["hn"]).astype(
            np.float32)
        c_next[brows, hcols] = np.asarray(res.results[ci]["cn"]).astype(
            np.float32)
    return (h_next, c_next), res


def kernel(**inputs):
    (h_next, c_next), _ = run(inputs, trace=False)
    return (h_next, c_next)
